# revision 19
# baseline (speedup 1.0000x reference)
"""AdaXbmTripletLoss kernel for 8 Trainium2 NeuronCores (Bass/Tile).

Math (see reference): loss = sum(hard * relu(d_ap + sqrt(margin) - d_an)) / count(hard)
with hard = ~is_nonneg & (sim > pos_sim - margin) & has_q.

Device strategy (per core, M sharded 8 ways -> ML=4096 rows):
  sim        = q @ rows^T                       (PE, bf16 inputs, f32 PSUM)
  d_an       = sqrt(alpha_b - 2*sim)            (ACT, per-partition bias, Sqrt)
  C_b       += sum_m 1[d_an < delta_b]          (DVE tensor_scalar is_lt, add-reduce accum)
  Smin_b    += sum_m min(d_an, delta_b)         (DVE tensor_scalar min, add-reduce accum)
Identity: sum_{mask} d_an = Smin - delta*(M - C), so
total_b = (gamma-delta)*C_b + (delta*M - Smin_b).
(The accum-variant HW instruction has a single embedded sync-wait slot, so each
accum op must depend on exactly one cross-engine producer - hence no ACT accum.)
where alpha_b = |q_b|^2 + 2*eps*sum(q_b) + D*eps^2, delta_b = sqrt(alpha_b - 2*thr_b),
thr_b = pos_sim_b - margin (delta_b = 0 if ~has_q).  The beta_m = |r_m|^2 - 2*eps*sum(r_m)
term is ~1 +- 1e-5 for L2-normalized rows; approximating it by 1 perturbs d_an by <4e-6.
Mask equivalence: d_an < delta  <=>  sim > thr (monotone map), so counts match the
reference's sim-space compare.

Host: total_b = (gamma_b - delta_b)*C_b + A_b with gamma_b = d_ap_b + sqrt(margin),
A_b = -negA_b (valid when delta_b <= gamma_b; rows where that fails are recomputed
exactly on host - never happens for this data).  The sparse is_nonneg correction
(expected ~900 (b,m) pairs out of 8.4M) is subtracted on host from exact f64 math.
"""

import os
import numpy as np
import ml_dtypes

B = 256
NCOL = 512
M = 32768
D = 512
K = 10
MARGIN = 0.1
EPS = 1e-6
TMARGIN = MARGIN ** 0.5
NCORES = 8
ML = M // NCORES          # 4096 rows per core
DCH = D // 128            # 4 contraction chunks
BT = B // 128             # 2 b-tiles
MT = ML // 512            # 8 m-tiles per core

_cache = {}
last_run = {}             # exec_time_ns etc. for test harness introspection


def _patch_tile_drain():
    """This container's walrus build allows only ONE embedded sync wait per
    instruction, but TileContext's kernel-tail drain aggregates a wait per
    logical proc (engines + DMA queues) onto a single Drain instruction ->
    'Too many sync wait commands'.  Replace it with standalone single-wait
    wait_ge instructions on the sync engine followed by a bare drain."""
    import concourse.tile as tile
    from concourse.tile_sem_assignment import tick_to_sem

    if getattr(tile.TileContext, "_drain_patched", False):
        return

    def _drain_and_barrier(self, tick_clock, wait_clock):
        gc = tick_clock.global_clock
        assert self.sems is not None
        for proc_idx, sem in sorted(self.sems.allocated().items()):
            tick = gc[proc_idx]
            if tick > 0:
                self.nc.sync.wait_ge(sem, tick_to_sem(tick, proc_idx))
        self.nc.sync.drain()
        self.nc.all_engine_barrier()
        popped = self.nc._tile_sem_poison_stack.pop()
        assert popped is self._sem_poison
        self.nc.clear_and_free_semaphores(list(self.sems.allocated().values()))
        self.nc.all_engine_barrier()

    tile.TileContext._drain_and_barrier = _drain_and_barrier
    tile.TileContext._drain_patched = True


def _build_nc():
    import concourse.bass as bass
    import concourse.mybir as mybir
    import concourse.tile as tile

    _patch_tile_drain()
    nc = bass.Bass()
    f32 = mybir.dt.float32
    bf16 = mybir.dt.bfloat16

    rows_ext = nc.declare_dram_parameter("rows_t", [DCH, 128, ML], bf16, False)
    q_ext = nc.declare_dram_parameter("q_t", [DCH, 128, B], bf16, False)
    # consts columns: alpha_bt0, alpha_bt1, delta_bt0, delta_bt1
    consts_ext = nc.declare_dram_parameter("consts", [128, 4], f32, False)
    # out columns: cnt_bt{0,1}, smin_bt{0,1}
    out_ext = nc.declare_dram_parameter("out", [128, 4], f32, True)

    with tile.TileContext(nc) as tc:
        with (
            tc.tile_pool(name="rows", bufs=1) as rows_pool,
            tc.tile_pool(name="qt", bufs=1) as qt_pool,
            tc.tile_pool(name="consts", bufs=1) as consts_pool,
            tc.tile_pool(name="psum", bufs=8, space="PSUM") as psum_pool,
            tc.tile_pool(name="dan", bufs=16) as dan_pool,
            tc.tile_pool(name="scr", bufs=32) as scr_pool,
            tc.tile_pool(name="cols", bufs=1) as cols_pool,
            tc.tile_pool(name="res", bufs=1) as res_pool,
        ):
            # Few DMA *instructions* (each still fans out across HW queues
            # internally): every DMA instruction occupies one DMAHW sem proc and
            # the kernel-tail Drain has a hard cap on embedded sync waits.
            rows_tile = rows_pool.tile([128, DCH, ML], bf16)
            rows_src = rows_ext.transpose([1, 0, 2])      # [128, DCH, ML] view
            half = ML // 2
            nc.sync.dma_start(rows_tile[:, :, 0:half], rows_src[:, :, 0:half])
            nc.sync.dma_start(rows_tile[:, :, half:ML], rows_src[:, :, half:ML])

            qt_tile = qt_pool.tile([128, DCH, B], bf16)
            nc.sync.dma_start(qt_tile[:], q_ext.transpose([1, 0, 2]))

            consts_tile = consts_pool.tile([128, 4], f32)
            nc.sync.dma_start(consts_tile[:], consts_ext[:])

            # Warm-up ops: absorb the consts-DMA semaphore wait on ACT/DVE so the
            # accum-variant instructions below only ever carry ONE sync wait
            # (the S3D3_AC struct has a single embedded wait slot).
            warm = consts_pool.tile([128, 2], f32)
            nc.scalar.copy(warm[:, 0:1], consts_tile[:, 0:1])
            nc.vector.tensor_scalar_add(warm[:, 1:2], consts_tile[:, 0:1], 0.0)

            cnt_cols = cols_pool.tile([128, BT, MT], f32)
            smin_cols = cols_pool.tile([128, BT, MT], f32)

            for bt in range(BT):
                alpha_ap = consts_tile[:, bt : bt + 1]
                delta_ap = consts_tile[:, 2 + bt : 3 + bt]
                for mt in range(MT):
                    psum = psum_pool.tile([128, 512], f32)
                    msl = slice(mt * 512, (mt + 1) * 512)
                    for k in range(DCH):
                        nc.tensor.matmul(
                            psum[:],
                            qt_tile[:, k, bt * 128 : (bt + 1) * 128],
                            rows_tile[:, k, msl],
                            start=(k == 0),
                            stop=(k == DCH - 1),
                        )
                    dan = dan_pool.tile([128, 512], f32)
                    # d_an = sqrt(-2*sim + alpha_b)
                    nc.scalar.activation(
                        dan[:], psum[:], mybir.ActivationFunctionType.Sqrt,
                        bias=alpha_ap, scale=-2.0,
                    )
                    scr1 = scr_pool.tile([128, 512], f32)
                    nc.vector.tensor_scalar(
                        scr1[:], dan[:], delta_ap, None,
                        op0=mybir.AluOpType.is_lt,
                        op1=mybir.AluOpType.add,
                        accum_out=cnt_cols[:, bt, mt : mt + 1],
                    )
                    scr2 = scr_pool.tile([128, 512], f32)
                    nc.vector.tensor_scalar(
                        scr2[:], dan[:], delta_ap, None,
                        op0=mybir.AluOpType.min,
                        op1=mybir.AluOpType.add,
                        accum_out=smin_cols[:, bt, mt : mt + 1],
                    )

            res = res_pool.tile([128, 4], f32)
            for bt in range(BT):
                nc.vector.tensor_reduce(
                    res[:, bt : bt + 1], cnt_cols[:, bt, :],
                    axis=mybir.AxisListType.X, op=mybir.AluOpType.add,
                )
                nc.vector.tensor_reduce(
                    res[:, 2 + bt : 3 + bt], smin_cols[:, bt, :],
                    axis=mybir.AxisListType.X, op=mybir.AluOpType.add,
                )
            nc.gpsimd.dma_start(out_ext[:], res[:])

    return nc


def _get_nc():
    if "nc" not in _cache:
        _cache["nc"] = _build_nc()
    return _cache["nc"]


def _install_ntff_hook():
    """The agent image's antenv lacks axon_hooks; shim it from trn_agent_boot so
    run_bass_kernel_spmd(trace=True) can capture NTFF profiles under axon."""
    import sys
    import types
    try:
        import antenv.axon_hooks  # noqa: F401
        return
    except ImportError:
        pass
    try:
        import antenv
        from trn_agent_boot.trn_boot import _ntff_profile_via_ctypes
        hook = {"h": _ntff_profile_via_ctypes("/opt/axon/libaxon_pjrt.so")}
        mod = types.ModuleType("antenv.axon_hooks")
        mod.get_axon_ntff_profile_hook = lambda: hook["h"]
        mod.set_axon_ntff_profile_hook = lambda h: hook.__setitem__("h", h)
        sys.modules["antenv.axon_hooks"] = mod
        antenv.axon_hooks = mod
    except Exception:
        pass


def kernel(inputs_col, inputs_row, targets_col, targets_row, qidxs, pidxs, nnegs, bs):
    from concourse.bass_utils import run_bass_kernel_spmd

    bs = int(np.asarray(bs))
    assert bs == B and inputs_row.shape == (M, D) and inputs_col.shape[1] == D

    inputs_col = np.asarray(inputs_col, dtype=np.float32)
    inputs_row = np.asarray(inputs_row, dtype=np.float32)
    targets_col = np.asarray(targets_col)
    targets_row = np.asarray(targets_row)
    qidxs = np.asarray(qidxs)
    nnegs = np.asarray(nnegs)

    q = inputs_col[:bs]                                        # [B, D] f32

    # ---- host-side index preprocessing (tiny int ops) ----
    match = targets_col[:bs, None] == qidxs[None, :]
    has_q = match.any(axis=1)
    qloc = match.argmax(axis=1)
    my_nnegs = nnegs[qloc]                                     # [B, K]

    pos_idx = bs + np.arange(bs)
    p = inputs_row[pos_idx]                                    # [B, D] f32

    # ---- per-query constants (f64 host math) ----
    q64 = q.astype(np.float64)
    p64 = p.astype(np.float64)
    na = (q64 * q64).sum(1)
    sa = q64.sum(1)
    # device d_an^2 = alpha - 2*sim, with beta_m = |r_m|^2 - 2*eps*sum(r_m) ~= 1
    # folded in (rows are L2-normalized), so alpha includes the +1.
    alpha = na + 2.0 * EPS * sa + D * EPS * EPS + 1.0
    d_ap = np.sqrt(((q64 - p64 + EPS) ** 2).sum(1))
    gamma = d_ap + TMARGIN
    pos_sim = (q64 * p64).sum(1)
    thr = pos_sim - MARGIN
    delta2 = alpha - 2.0 * thr
    delta = np.sqrt(np.maximum(delta2, 0.0))
    delta = np.where(has_q, delta, 0.0)
    # rows where the (gamma - delta)*C + A identity breaks -> exact host fallback
    bad_b = np.flatnonzero(has_q & (delta > gamma))

    # ---- device inputs ----
    rows_t = np.ascontiguousarray(inputs_row.T).astype(ml_dtypes.bfloat16)  # [D, M]
    rows_t = rows_t.reshape(DCH, 128, M)
    q_t = np.ascontiguousarray(q.T).astype(ml_dtypes.bfloat16).reshape(DCH, 128, B)
    consts = np.empty((128, 4), np.float32)
    consts[:, 0] = alpha[:128]
    consts[:, 1] = alpha[128:]
    consts[:, 2] = delta[:128]
    consts[:, 3] = delta[128:]

    in_maps = []
    for c in range(NCORES):
        in_maps.append({
            "rows_t": np.ascontiguousarray(rows_t[:, :, c * ML : (c + 1) * ML]),
            "q_t": q_t,
            "consts": consts,
        })

    nc = _get_nc()
    trace = bool(os.environ.get("ATHENA_KERNEL_TRACE"))
    if trace:
        _install_ntff_hook()
    r = run_bass_kernel_spmd(nc, in_maps, list(range(NCORES)), trace=trace)
    last_run["exec_time_ns"] = r.exec_time_ns
    last_run["results"] = r

    # ---- gather partials ----
    cnt = np.zeros(B, np.float64)
    smin = np.zeros(B, np.float64)
    for c in range(NCORES):
        o = np.asarray(r.results[c]["out"], dtype=np.float64)  # [128, 4]
        cnt[:128] += o[:, 0]
        cnt[128:] += o[:, 1]
        smin[:128] += o[:, 2]
        smin[128:] += o[:, 3]
    # sum_{mask} d_an = Smin - delta*(M - C)  =>  total = (g-d)*C + d*M - Smin
    total_b = (gamma - delta) * cnt + (delta * M - smin)
    count_b = cnt

    # ---- exact host fallback for rows violating delta <= gamma ----
    rows64 = None
    if len(bad_b):
        rows64 = inputs_row.astype(np.float64)
        nb_all = (rows64 * rows64).sum(1)
        sb_all = rows64.sum(1)
        for b in bad_b:
            simrow = rows64 @ q64[b]
            mask = simrow > thr[b]
            d2 = (na[b] + nb_all - 2.0 * simrow
                  + 2.0 * EPS * (sa[b] - sb_all) + D * EPS * EPS)
            d_an = np.sqrt(np.maximum(d2, 0.0))
            count_b[b] = mask.sum()
            total_b[b] = np.maximum(gamma[b] - d_an, 0.0)[mask].sum()

    # ---- sparse is_nonneg correction (host, exact) ----
    order = np.argsort(targets_row, kind="stable")
    tr_sorted = targets_row[order]
    lo = np.searchsorted(tr_sorted, my_nnegs.ravel(), side="left")
    hi = np.searchsorted(tr_sorted, my_nnegs.ravel(), side="right")
    pairs = set()
    for flat, (l, h) in enumerate(zip(lo, hi)):
        if h > l:
            b = flat // K
            if has_q[b]:
                for m in order[l:h]:
                    pairs.add((b, int(m)))
    if pairs:
        pb = np.fromiter((x[0] for x in pairs), np.int64, len(pairs))
        pm = np.fromiter((x[1] for x in pairs), np.int64, len(pairs))
        rows_sel = inputs_row[pm].astype(np.float64)
        sims = (q64[pb] * rows_sel).sum(1)
        sel = sims > thr[pb]
        pb, pm, sims, rows_sel = pb[sel], pm[sel], sims[sel], rows_sel[sel]
        nb = (rows_sel * rows_sel).sum(1)
        sb = rows_sel.sum(1)
        d2 = na[pb] + nb - 2.0 * sims + 2.0 * EPS * (sa[pb] - sb) + D * EPS * EPS
        d_an = np.sqrt(np.maximum(d2, 0.0))
        tl = np.maximum(gamma[pb] - d_an, 0.0)
        np.add.at(count_b, pb, -1.0)
        np.add.at(total_b, pb, -tl)

    neg_count = count_b.sum()
    total = total_b.sum()
    loss = total / neg_count if neg_count > 0 else 0.0
    return np.float32(loss)


# revision 26
# speedup vs baseline: 1.1984x; 1.1984x over previous
"""AdaXbmTripletLoss kernel for 8 Trainium2 NeuronCores (Bass/Tile).

Math (see reference): loss = sum(hard * relu(d_ap + sqrt(margin) - d_an)) / count(hard)
with hard = ~is_nonneg & (sim > pos_sim - margin) & has_q.

Device strategy (per core, M sharded 8 ways -> ML=4096 rows):
  sim        = q @ rows^T                       (PE, bf16 inputs, f32 PSUM)
  d_an       = sqrt(alpha_b - 2*sim)            (ACT, per-partition bias, Sqrt)
  C_b       += sum_m 1[d_an < delta_b]          (DVE tensor_scalar is_lt, add-reduce accum)
  Smin_b    += sum_m min(d_an, delta_b)         (DVE tensor_scalar min, add-reduce accum)
Identity: sum_{mask} d_an = Smin - delta*(M - C), so
total_b = (gamma-delta)*C_b + (delta*M - Smin_b).
(The accum-variant HW instruction has a single embedded sync-wait slot, so each
accum op must depend on exactly one cross-engine producer - hence no ACT accum.)
where alpha_b = |q_b|^2 + 2*eps*sum(q_b) + D*eps^2, delta_b = sqrt(alpha_b - 2*thr_b),
thr_b = pos_sim_b - margin (delta_b = 0 if ~has_q).  The beta_m = |r_m|^2 - 2*eps*sum(r_m)
term is ~1 +- 1e-5 for L2-normalized rows; approximating it by 1 perturbs d_an by <4e-6.
Mask equivalence: d_an < delta  <=>  sim > thr (monotone map), so counts match the
reference's sim-space compare.

Host: total_b = (gamma_b - delta_b)*C_b + A_b with gamma_b = d_ap_b + sqrt(margin),
A_b = -negA_b (valid when delta_b <= gamma_b; rows where that fails are recomputed
exactly on host - never happens for this data).  The sparse is_nonneg correction
(expected ~900 (b,m) pairs out of 8.4M) is subtracted on host from exact f64 math.
"""

import os
import numpy as np
import ml_dtypes

B = 256
NCOL = 512
M = 32768
D = 512
K = 10
MARGIN = 0.1
EPS = 1e-6
TMARGIN = MARGIN ** 0.5
NCORES = 8
ML = M // NCORES          # 4096 rows per core
DCH = D // 128            # 4 contraction chunks
BT = B // 128             # 2 b-tiles
MT = ML // 512            # 8 m-tiles per core

_cache = {}
last_run = {}             # exec_time_ns etc. for test harness introspection


def _patch_tile_drain():
    """This container's walrus build allows only ONE embedded sync wait per
    instruction, but TileContext's kernel-tail drain aggregates a wait per
    logical proc (engines + DMA queues) onto a single Drain instruction ->
    'Too many sync wait commands'.  Replace it with standalone single-wait
    wait_ge instructions on the sync engine followed by a bare drain."""
    import concourse.tile as tile
    from concourse.tile_sem_assignment import tick_to_sem

    if getattr(tile.TileContext, "_drain_patched", False):
        return

    def _drain_and_barrier(self, tick_clock, wait_clock):
        gc = tick_clock.global_clock
        assert self.sems is not None
        for proc_idx, sem in sorted(self.sems.allocated().items()):
            tick = gc[proc_idx]
            if tick > 0:
                self.nc.sync.wait_ge(sem, tick_to_sem(tick, proc_idx))
        self.nc.sync.drain()
        self.nc.all_engine_barrier()
        popped = self.nc._tile_sem_poison_stack.pop()
        assert popped is self._sem_poison
        self.nc.clear_and_free_semaphores(list(self.sems.allocated().values()))
        self.nc.all_engine_barrier()

    tile.TileContext._drain_and_barrier = _drain_and_barrier
    tile.TileContext._drain_patched = True


def _build_nc():
    import concourse.bass as bass
    import concourse.mybir as mybir
    import concourse.tile as tile

    _patch_tile_drain()
    nc = bass.Bass()
    f32 = mybir.dt.float32
    bf16 = mybir.dt.bfloat16

    # rows relayout: [G groups of 1024 m][128 partitions (d within chunk)]
    # [DCH chunks][1024 m] -> per-partition contiguous run = DCH*GM*2 = 8KB.
    GM = 1024                 # m per group
    G = ML // GM              # 4 groups per core
    rows_ext = nc.declare_dram_parameter("rows_t", [G, 128, DCH, GM], bf16, False)
    q_ext = nc.declare_dram_parameter("q_t", [DCH, 128, B], bf16, False)
    # consts columns: alpha_bt0, alpha_bt1, delta_bt0, delta_bt1
    consts_ext = nc.declare_dram_parameter("consts", [128, 4], f32, False)
    # out columns: cnt_bt{0,1}, smin_bt{0,1}
    out_ext = nc.declare_dram_parameter("out", [128, 4], f32, True)

    with tile.TileContext(nc) as tc:
        with (
            tc.tile_pool(name="rows", bufs=1) as rows_pool,
            tc.tile_pool(name="qt", bufs=1) as qt_pool,
            tc.tile_pool(name="consts", bufs=1) as consts_pool,
            tc.tile_pool(name="psum", bufs=4, space="PSUM") as psum_pool,
            tc.tile_pool(name="dan", bufs=BT * G) as dan_pool,
            tc.tile_pool(name="scr", bufs=2 * BT * G) as scr_pool,
            tc.tile_pool(name="cols", bufs=1) as cols_pool,
            tc.tile_pool(name="res", bufs=1) as res_pool,
        ):
            # small inputs first so their queues drain early
            qt_tile = qt_pool.tile([128, DCH, B], bf16)
            nc.sync.dma_start(qt_tile[:], q_ext.transpose([1, 0, 2]))
            consts_tile = consts_pool.tile([128, 4], f32)
            nc.sync.dma_start(consts_tile[:], consts_ext[:])

            # one DMA per m-group; each group feeds its own matmul batch so
            # PE starts after ~1MB instead of after the full 4MB
            rows_tiles = []
            for g in range(G):
                rt = rows_pool.tile([128, DCH, GM], bf16, tag=f"rows{g}")
                nc.sync.dma_start(rt[:], rows_ext[g])
                rows_tiles.append(rt)

            # Warm-up ops: absorb the consts-DMA wait on ACT/DVE (accum-variant
            # instructions have a single embedded sync-wait slot) and pull the
            # ACT Sqrt table load off the critical path.
            warm = consts_pool.tile([128, 2], f32)
            nc.scalar.activation(
                warm[:, 0:1], consts_tile[:, 0:1],
                mybir.ActivationFunctionType.Sqrt,
            )
            nc.vector.tensor_scalar_add(warm[:, 1:2], consts_tile[:, 0:1], 0.0)

            cnt_cols = cols_pool.tile([128, BT, G], f32)
            smin_cols = cols_pool.tile([128, BT, G], f32)

            # g outer so each 1MB group is fully consumed (both b-tiles)
            # before the next group's DMA must have landed
            for g in range(G):
                # dummy weight load absorbs the rows-DMA wait on the PE queue so
                # the group's first real matmul stays under the 1-wait limit
                nc.tensor.ldweights(rows_tiles[g][:, 0, 0:1])
                for bt in range(BT):
                    alpha_ap = consts_tile[:, bt : bt + 1]
                    delta_ap = consts_tile[:, 2 + bt : 3 + bt]
                    psum = psum_pool.tile([128, GM], f32)  # 2 banks
                    for k in range(DCH):
                        lhs = qt_tile[:, k, bt * 128 : (bt + 1) * 128]
                        for h in range(GM // 512):
                            hsl = slice(h * 512, (h + 1) * 512)
                            nc.tensor.matmul(
                                psum[:, hsl],
                                lhs,
                                rows_tiles[g][:, k, hsl],
                                start=(k == 0),
                                stop=(k == DCH - 1),
                            )
                    dan = dan_pool.tile([128, GM], f32)
                    # d_an = sqrt(-2*sim + alpha_b)
                    nc.scalar.activation(
                        dan[:], psum[:], mybir.ActivationFunctionType.Sqrt,
                        bias=alpha_ap, scale=-2.0,
                    )
                    scr1 = scr_pool.tile([128, GM], f32)
                    nc.vector.tensor_scalar(
                        scr1[:], dan[:], delta_ap, None,
                        op0=mybir.AluOpType.is_lt,
                        op1=mybir.AluOpType.add,
                        accum_out=cnt_cols[:, bt, g : g + 1],
                    )
                    scr2 = scr_pool.tile([128, GM], f32)
                    nc.vector.tensor_scalar(
                        scr2[:], dan[:], delta_ap, None,
                        op0=mybir.AluOpType.min,
                        op1=mybir.AluOpType.add,
                        accum_out=smin_cols[:, bt, g : g + 1],
                    )

            res = res_pool.tile([128, 4], f32)
            for bt in range(BT):
                nc.vector.tensor_reduce(
                    res[:, bt : bt + 1], cnt_cols[:, bt, :],
                    axis=mybir.AxisListType.X, op=mybir.AluOpType.add,
                )
                nc.vector.tensor_reduce(
                    res[:, 2 + bt : 3 + bt], smin_cols[:, bt, :],
                    axis=mybir.AxisListType.X, op=mybir.AluOpType.add,
                )
            nc.gpsimd.dma_start(out_ext[:], res[:])

    # Post-pass: matmuls that evict a PSUM slot carry two waits - the evicting
    # reader's ACT wait plus a same-engine PE wait that the ACT wait transitively
    # implies (the sqrt at that ACT tick itself waited for those PE matmuls;
    # semaphores are monotone).  The walrus build allows one embedded sync wait,
    # so drop the redundant PE self-wait.
    for bb in nc.m.functions[0].blocks:
        for i in bb.instructions:
            si = i.sync_info
            if si is None or type(i).__name__ != "InstMatmult":
                continue
            w = si.on_wait
            if len(w) >= 2 and any(x.ant_name.startswith("Activation") for x in w):
                keep = [x for x in w if not x.ant_name.startswith("PE_")]
                if len(keep) < len(w) and len(keep) == 1:
                    si.on_wait = keep

    return nc


def _get_nc():
    if "nc" not in _cache:
        _cache["nc"] = _build_nc()
    return _cache["nc"]


def _install_ntff_hook():
    """The agent image's antenv lacks axon_hooks; shim it from trn_agent_boot so
    run_bass_kernel_spmd(trace=True) can capture NTFF profiles under axon."""
    import sys
    import types
    try:
        import antenv.axon_hooks  # noqa: F401
        return
    except ImportError:
        pass
    try:
        import antenv
        from trn_agent_boot.trn_boot import _ntff_profile_via_ctypes
        hook = {"h": _ntff_profile_via_ctypes("/opt/axon/libaxon_pjrt.so")}
        mod = types.ModuleType("antenv.axon_hooks")
        mod.get_axon_ntff_profile_hook = lambda: hook["h"]
        mod.set_axon_ntff_profile_hook = lambda h: hook.__setitem__("h", h)
        sys.modules["antenv.axon_hooks"] = mod
        antenv.axon_hooks = mod
    except Exception:
        pass


def kernel(inputs_col, inputs_row, targets_col, targets_row, qidxs, pidxs, nnegs, bs):
    from concourse.bass_utils import run_bass_kernel_spmd

    bs = int(np.asarray(bs))
    assert bs == B and inputs_row.shape == (M, D) and inputs_col.shape[1] == D

    inputs_col = np.asarray(inputs_col, dtype=np.float32)
    inputs_row = np.asarray(inputs_row, dtype=np.float32)
    targets_col = np.asarray(targets_col)
    targets_row = np.asarray(targets_row)
    qidxs = np.asarray(qidxs)
    nnegs = np.asarray(nnegs)

    q = inputs_col[:bs]                                        # [B, D] f32

    # ---- host-side index preprocessing (tiny int ops) ----
    match = targets_col[:bs, None] == qidxs[None, :]
    has_q = match.any(axis=1)
    qloc = match.argmax(axis=1)
    my_nnegs = nnegs[qloc]                                     # [B, K]

    pos_idx = bs + np.arange(bs)
    p = inputs_row[pos_idx]                                    # [B, D] f32

    # ---- per-query constants (f64 host math) ----
    q64 = q.astype(np.float64)
    p64 = p.astype(np.float64)
    na = (q64 * q64).sum(1)
    sa = q64.sum(1)
    # device d_an^2 = alpha - 2*sim, with beta_m = |r_m|^2 - 2*eps*sum(r_m) ~= 1
    # folded in (rows are L2-normalized), so alpha includes the +1.
    alpha = na + 2.0 * EPS * sa + D * EPS * EPS + 1.0
    d_ap = np.sqrt(((q64 - p64 + EPS) ** 2).sum(1))
    gamma = d_ap + TMARGIN
    pos_sim = (q64 * p64).sum(1)
    thr = pos_sim - MARGIN
    delta2 = alpha - 2.0 * thr
    delta = np.sqrt(np.maximum(delta2, 0.0))
    delta = np.where(has_q, delta, 0.0)
    # rows where the (gamma - delta)*C + A identity breaks -> exact host fallback
    bad_b = np.flatnonzero(has_q & (delta > gamma))

    # ---- device inputs ----
    # rows_t device layout per core: [G, 128, DCH, GM] where
    # rows_t[g, p, k, m] = inputs_row[c*ML + g*GM + m, k*128 + p]
    GM = 1024
    G = ML // GM
    rt = inputs_row.T.astype(ml_dtypes.bfloat16)            # [D, M]
    rt = rt.reshape(DCH, 128, NCORES, G, GM)                # k, p, c, g, m
    q_t = np.ascontiguousarray(q.T).astype(ml_dtypes.bfloat16).reshape(DCH, 128, B)
    consts = np.empty((128, 4), np.float32)
    consts[:, 0] = alpha[:128]
    consts[:, 1] = alpha[128:]
    consts[:, 2] = delta[:128]
    consts[:, 3] = delta[128:]

    in_maps = []
    for c in range(NCORES):
        in_maps.append({
            "rows_t": np.ascontiguousarray(rt[:, :, c].transpose(2, 1, 0, 3)),
            "q_t": q_t,
            "consts": consts,
        })

    nc = _get_nc()
    trace = bool(os.environ.get("ATHENA_KERNEL_TRACE"))
    if trace:
        _install_ntff_hook()
    r = run_bass_kernel_spmd(nc, in_maps, list(range(NCORES)), trace=trace)
    last_run["exec_time_ns"] = r.exec_time_ns
    last_run["results"] = r

    # ---- gather partials ----
    cnt = np.zeros(B, np.float64)
    smin = np.zeros(B, np.float64)
    for c in range(NCORES):
        o = np.asarray(r.results[c]["out"], dtype=np.float64)  # [128, 4]
        cnt[:128] += o[:, 0]
        cnt[128:] += o[:, 1]
        smin[:128] += o[:, 2]
        smin[128:] += o[:, 3]
    # sum_{mask} d_an = Smin - delta*(M - C)  =>  total = (g-d)*C + d*M - Smin
    total_b = (gamma - delta) * cnt + (delta * M - smin)
    count_b = cnt

    # ---- exact host fallback for rows violating delta <= gamma ----
    rows64 = None
    if len(bad_b):
        rows64 = inputs_row.astype(np.float64)
        nb_all = (rows64 * rows64).sum(1)
        sb_all = rows64.sum(1)
        for b in bad_b:
            simrow = rows64 @ q64[b]
            mask = simrow > thr[b]
            d2 = (na[b] + nb_all - 2.0 * simrow
                  + 2.0 * EPS * (sa[b] - sb_all) + D * EPS * EPS)
            d_an = np.sqrt(np.maximum(d2, 0.0))
            count_b[b] = mask.sum()
            total_b[b] = np.maximum(gamma[b] - d_an, 0.0)[mask].sum()

    # ---- sparse is_nonneg correction (host, exact) ----
    order = np.argsort(targets_row, kind="stable")
    tr_sorted = targets_row[order]
    lo = np.searchsorted(tr_sorted, my_nnegs.ravel(), side="left")
    hi = np.searchsorted(tr_sorted, my_nnegs.ravel(), side="right")
    pairs = set()
    for flat, (l, h) in enumerate(zip(lo, hi)):
        if h > l:
            b = flat // K
            if has_q[b]:
                for m in order[l:h]:
                    pairs.add((b, int(m)))
    if pairs:
        pb = np.fromiter((x[0] for x in pairs), np.int64, len(pairs))
        pm = np.fromiter((x[1] for x in pairs), np.int64, len(pairs))
        rows_sel = inputs_row[pm].astype(np.float64)
        sims = (q64[pb] * rows_sel).sum(1)
        sel = sims > thr[pb]
        pb, pm, sims, rows_sel = pb[sel], pm[sel], sims[sel], rows_sel[sel]
        nb = (rows_sel * rows_sel).sum(1)
        sb = rows_sel.sum(1)
        d2 = na[pb] + nb - 2.0 * sims + 2.0 * EPS * (sa[pb] - sb) + D * EPS * EPS
        d_an = np.sqrt(np.maximum(d2, 0.0))
        tl = np.maximum(gamma[pb] - d_an, 0.0)
        np.add.at(count_b, pb, -1.0)
        np.add.at(total_b, pb, -tl)

    neg_count = count_b.sum()
    total = total_b.sum()
    loss = total / neg_count if neg_count > 0 else 0.0
    return np.float32(loss)
